# revision 54
# baseline (speedup 1.0000x reference)
"""Multi-head causal self-attention (B=64, T=256, C=384, H=6) on 8 NeuronCores.

Data-parallel over batch: each core processes 8 batches (2048 tokens).
Layouts (all chosen so no device-side transposes are needed):
  - xT, Q.T, K.T feature-major [C, tokens]
  - V token-major [tokens, 128*H]: per head 64 V columns + 64 ones columns,
    so each O.T matmul (stationary [128tok, 128]) produces O.T on PSUM
    partitions 0:64 and the softmax denominator Z replicated on partitions
    64:128 — no separate Z matmuls and no partition-broadcast needed
  - scores computed transposed (S.T[tk, tq]); causal mask via gpsimd
    affine_select on the exp'd probabilities (as in the original module)
  - per pair, one f32 copy evacuates the O PSUM bank to SBUF; 1/Z and the
    normalization then run batched per-batch on vector/gpsimd from SBUF
  - catT merged per 512-token chunk ([128, 1536], feature-chunk-major)
Pipeline: QK-proj(t) / V-proj(t) / scores(u) / O(u-1) interleaved in
emission order (one-stage software pipeline) so the PE stream stays dense;
the out-projection is emitted as soon as its token chunk is normalized.
Matmul operands are bf16; accumulation and softmax denominators fp32;
output is written bf16 (tolerance 2e-2 leaves ample margin) and upcast on
the host.
"""

import sys

import ml_dtypes
import numpy as np

for _p in ("/opt/trn_rl_repo", "/root/.axon_site/_ro/trn_rl_repo"):
    if _p not in sys.path:
        sys.path.insert(0, _p)

import concourse.bass as bass
import concourse.tile as tile
from concourse import bacc, mybir
from concourse.bass_utils import run_bass_kernel_spmd

B, T, C, H, D = 64, 256, 384, 6, 64
NCORES = 8
BB = B // NCORES  # batches per core = 8
TOK = BB * T      # tokens per core = 2048
SCALE = float(C) ** -0.5
F32 = mybir.dt.float32
BF16 = mybir.dt.bfloat16
NPBF = ml_dtypes.bfloat16

NT4 = TOK // 512  # 4 column-chunks of 512 tokens (2 batches each)
NKC = C // 128    # 3 chunks of 128 over feature dim


def build_module():
    nc = bacc.Bacc("TRN2", target_bir_lowering=False, debug=False)

    xT = nc.dram_tensor("xT", [C, TOK], BF16, kind="ExternalInput").ap()
    wall = nc.dram_tensor("wall", [C, 4 * C], BF16, kind="ExternalInput").ap()
    wobc = nc.dram_tensor("wobc", [C, 1], F32, kind="ExternalInput").ap()
    yT = nc.dram_tensor("yT", [C, TOK], BF16, kind="ExternalOutput").ap()

    with tile.TileContext(nc) as tc:
        import contextlib

        ctx = contextlib.ExitStack()
        with ctx:
            consts = ctx.enter_context(tc.tile_pool(name="consts", bufs=1))

            def ptile(name, shape, dt=BF16):
                return consts.tile(shape, dt, tag=name, name=name)

            # ---- persistent SBUF tiles ----
            w_sb = [[ptile(f"w{j}_{k}", [128, C]) for k in range(NKC)]
                    for j in range(4)]  # q, k, v, o
            wq_sb, wk_sb, wv_sb, wo_sb = w_sb
            wob_sb = [ptile(f"wob{k}", [128, 1], F32) for k in range(NKC)]
            xt_sb = [ptile(f"xt{k}", [128, TOK]) for k in range(NKC)]
            qt_sb = [[ptile(f"qt{k}_{t}", [128, 512]) for t in range(NT4)] for k in range(NKC)]
            kt_sb = [[ptile(f"kt{k}_{t}", [128, 512]) for t in range(NT4)] for k in range(NKC)]
            cat_sb = [ptile(f"cat{t}", [128, 3 * 512]) for t in range(NT4)]
            v_sb = [ptile(f"v{t}", [128, 128 * H]) for t in range(2 * BB)]
            warm_sb = ptile("warm", [128, 512])

            # ---- PSUM pools (8 banks: pa 2 + ps 2x2 + po 2) ----
            pa = ctx.enter_context(tc.tile_pool(name="pa", bufs=2, space="PSUM"))
            ps = ctx.enter_context(tc.tile_pool(name="ps", bufs=2, space="PSUM"))
            po = ctx.enter_context(tc.tile_pool(name="po", bufs=2, space="PSUM"))

            # ---- HAM warm-up first (before anything else queues on
            # vector/tensor): dummy matmuls so the PE clock is at 8/8 by the
            # time the first real operands land in SBUF
            nc.gpsimd.memset(warm_sb, 0.0)
            pwarm = pa.tile([128, 512], F32, tag="pa", name="pwarm")
            for i in range(16):
                nc.tensor.matmul(pwarm, warm_sb[:, 0:128], warm_sb,
                                 start=True, stop=True, skip_group_check=True)

            def emit_filler(tag, n=3):
                pf = pa.tile([128, 512], F32, tag="pa", name=f"fill{tag}")
                for i in range(n):
                    nc.tensor.matmul(pf, warm_sb[:, 0:128], warm_sb,
                                     start=True, stop=True,
                                     skip_group_check=True)

            # ---- input DMAs: split fine-grained, ordered so the first QK
            # matmuls can start as early as possible
            dma_list = []
            for j, c0 in ((0, 0), (1, C)):  # wq, wk
                for k in range(NKC):
                    dma_list.append((w_sb[j][k], wall[128 * k:128 * (k + 1), c0:c0 + C]))
            for k in range(NKC):
                dma_list.append((xt_sb[k][:, 0:512], xT[128 * k:128 * (k + 1), 0:512]))
            for k in range(NKC):
                dma_list.append((w_sb[2][k], wall[128 * k:128 * (k + 1), 2 * C:3 * C]))
            for k in range(NKC):
                dma_list.append((xt_sb[k][:, 512:1024], xT[128 * k:128 * (k + 1), 512:1024]))
            for k in range(NKC):
                dma_list.append((w_sb[3][k], wall[128 * k:128 * (k + 1), 3 * C:4 * C]))
            for t in range(2, NT4):
                for k in range(NKC):
                    dma_list.append((
                        xt_sb[k][:, 512 * t:512 * (t + 1)],
                        xT[128 * k:128 * (k + 1), 512 * t:512 * (t + 1)],
                    ))
            for k in range(NKC):
                dma_list.append((wob_sb[k], wobc[128 * k:128 * (k + 1), :]))
            # first-needed 9 transfers strict round-robin (minimize time to
            # first real matmul); the rest weighted sync-heavy
            in_queues = ([nc.sync, nc.gpsimd, nc.scalar] * 3
                         + [nc.sync, nc.gpsimd, nc.sync, nc.scalar,
                            nc.sync, nc.gpsimd, nc.gpsimd, nc.sync] * 3)
            for i, (dst, src) in enumerate(dma_list):
                in_queues[i % len(in_queues)].dma_start(out=dst, in_=src)

            pt_pool = ctx.enter_context(tc.tile_pool(name="ptp", bufs=BB * H // 2))
            ou_pool = ctx.enter_context(tc.tile_pool(name="oup", bufs=3))
            rz_pool = ctx.enter_context(tc.tile_pool(name="rzp", bufs=3))
            y_pool = ctx.enter_context(tc.tile_pool(name="yp", bufs=3))

            # ones columns FIRST in each head's V block: the O matmul then
            # writes Z (replicated) to PSUM partitions 0:64 (base-0, readable
            # by the custom-DVE reciprocal) and O.T to partitions 64:128
            for tb in range(2 * BB):
                vv = v_sb[tb].rearrange("p (h w) -> p h w", w=2 * D)
                nc.gpsimd.memset(vv[:, :, 0:D], 1.0)

            # ---------------- emission helpers ----------------
            def emit_qk(t, co):
                for which, wsb, outsb, eng in (
                    (0, wq_sb, qt_sb, nc.scalar),
                    (1, wk_sb, kt_sb, nc.vector),
                ):
                    pqk = pa.tile([128, 512], F32, tag="pa", name=f"pqk{which}{co}_{t}")
                    for kc in range(NKC):
                        nc.tensor.matmul(
                            pqk,
                            wsb[kc][:, 128 * co:128 * (co + 1)],
                            xt_sb[kc][:, 512 * t:512 * (t + 1)],
                            start=(kc == 0),
                            stop=(kc == NKC - 1),
                        )
                    if eng is nc.scalar:
                        eng.copy(outsb[co][t], pqk)
                    else:
                        eng.tensor_copy(outsb[co][t], pqk)

            def emit_v(tb):
                pv = pa.tile([128, C], F32, tag="pa", name=f"pv{tb}")
                for kc in range(NKC):
                    nc.tensor.matmul(
                        pv,
                        xt_sb[kc][:, 128 * tb:128 * (tb + 1)],
                        wv_sb[kc],
                        start=(kc == 0),
                        stop=(kc == NKC - 1),
                    )
                vv = v_sb[tb].rearrange("p (h w) -> p h w", w=2 * D)
                if tb % 2 == 0:
                    nc.scalar.copy(
                        vv[:, :, D:2 * D], pv.rearrange("p (h d) -> p h d", d=D)
                    )
                else:
                    nc.vector.tensor_copy(
                        vv[:, :, D:2 * D], pv.rearrange("p (h d) -> p h d", d=D)
                    )

            def emit_scores(u):
                b, hp = u
                t, qc = b // 2, (b % 2) * 256
                p_s = ps.tile([128, 1024], F32, tag="ps", name=f"s{b}_{hp}")
                qt, kt = qt_sb[hp][t], kt_sb[hp][t]
                for tkb in range(2):  # interleave heads for row-tile overlap
                    for hh in range(2):
                        r0, s0 = 64 * hh, 512 * hh
                        if tkb == 0:
                            nc.tensor.matmul(
                                p_s[:, s0:s0 + 256],
                                kt[r0:r0 + 64, qc:qc + 128],
                                qt[r0:r0 + 64, qc:qc + 256],
                                start=True, stop=True,
                            )
                        else:
                            nc.tensor.matmul(
                                p_s[:, s0 + 256:s0 + 384],
                                kt[r0:r0 + 64, qc + 128:qc + 256],
                                qt[r0:r0 + 64, qc + 128:qc + 256],
                                start=True, stop=True,
                            )
                # exp (scalar) then causal mask on diagonal blocks (gpsimd)
                pt = pt_pool.tile([128, 768], BF16, tag="pt", name=f"pt{b}_{hp}")
                nc.scalar.activation(
                    pt.rearrange("p (a q) -> p a q", q=384),
                    p_s.rearrange("p (a q) -> p a q", q=512)[:, :, 0:384],
                    mybir.ActivationFunctionType.Exp, scale=SCALE,
                )
                for hh in range(2):
                    sel = pt[:, 384 * hh:384 * (hh + 1)] \
                        .rearrange("p (c i) -> p c i", i=128)[:, 0::2, :]
                    nc.gpsimd.affine_select(
                        out=sel, in_=sel,
                        pattern=[[0, 2], [1, 128]],
                        compare_op=mybir.AluOpType.is_ge,
                        fill=0.0, base=0, channel_multiplier=-1,
                    )
                return pt

            def emit_o(u, pt):
                b, hp = u
                t, qc = b // 2, (b % 2) * 256
                p_o = po.tile([128, 512], F32, tag="po", name=f"po{b}_{hp}")
                for hh in range(2):
                    h = 2 * hp + hh
                    c0 = 256 * hh
                    nc.tensor.matmul(
                        p_o[:, c0:c0 + 256],
                        v_sb[2 * b][:, 128 * h:128 * (h + 1)],
                        pt[:, 384 * hh:384 * hh + 256],
                        start=True, stop=False, skip_group_check=True,
                    )
                    nc.tensor.matmul(
                        p_o[:, c0 + 128:c0 + 256],
                        v_sb[2 * b + 1][:, 128 * h:128 * (h + 1)],
                        pt[:, 384 * hh + 256:384 * hh + 384],
                        start=False, stop=True, skip_group_check=True,
                    )
                # 1/Z straight off PSUM (base-0), O.T rows to SBUF (bf16)
                rz = rz_pool.tile([64, 512], F32, tag="rz", name=f"rz{b}_{hp}")
                nc.vector.reciprocal_approx_fast(rz, p_o[0:64, :])
                ou = ou_pool.tile([64, 512], BF16, tag="ou", name=f"ou{b}_{hp}")
                nc.scalar.copy(ou, p_o[64:128, :])
                # normalize into catT (all-SBUF base-0 inputs; split engines)
                for hh, eng in ((0, nc.gpsimd), (1, nc.vector)):
                    c0 = 256 * hh
                    eng.tensor_mul(
                        cat_sb[t][64 * hh:64 * (hh + 1),
                                  512 * hp + qc:512 * hp + qc + 256],
                        ou[:, c0:c0 + 256],
                        rz[:, c0:c0 + 256],
                    )

            def emit_p3(t, qc0, n, bias_eng="vector"):
                for co in range(NKC):
                    pyk = pa.tile([128, n], F32, tag="pa", name=f"py{co}_{t}_{qc0}")
                    for kc in range(NKC):
                        nc.tensor.matmul(
                            pyk,
                            wo_sb[kc][:, 128 * co:128 * (co + 1)],
                            cat_sb[t][:, 512 * kc + qc0:512 * kc + qc0 + n],
                            start=(kc == 0),
                            stop=(kc == NKC - 1),
                        )
                    yt = y_pool.tile([128, n], BF16, tag="yt", name=f"yt{co}_{t}_{qc0}")
                    if bias_eng == "vector":
                        nc.vector.tensor_scalar_add(yt, pyk, wob_sb[co][:, 0:1])
                    else:
                        nc.scalar.activation(
                            yt, pyk, mybir.ActivationFunctionType.Identity,
                            bias=wob_sb[co][:, 0:1],
                        )
                    q = nc.sync if co < 2 else nc.scalar
                    q.dma_start(
                        out=yT[128 * co:128 * (co + 1),
                               512 * t + qc0:512 * t + qc0 + n],
                        in_=yt,
                    )

            # ---------------- main pipeline ----------------
            units = [(b, hp) for b in range(BB) for hp in range(H // 2)]
            # two-stage software pipeline: O(u) is emitted two units after
            # scores(u), hiding the exp->mask cross-engine latency
            pend = []
            emit_qk(0, 0)
            emit_qk(0, 1)
            for i, u in enumerate(units):
                # scores + exp first (earliest start for the exp->mask chain)
                pt = emit_scores(u)
                # QK chunk for unit i+2 (two units of slack before use) and
                # this unit's share of the V projection
                t, j = i // 6, i % 6
                t2, j2 = (i + 2) // 6, (i + 2) % 6
                if i + 2 < len(units) and j2 < 3:
                    emit_qk(t2, j2)
                if j < 2:
                    emit_v(4 * t + 2 * j)
                    emit_v(4 * t + 2 * j + 1)
                if len(pend) == 3:
                    emit_o(*pend.pop(0))
                # out-proj for chunk t four units after its last norm
                if i >= 11 and (i - 11) % 6 == 0:
                    emit_p3((i - 11) // 6, 0, 512)
                pend.append((u, pt))
            # drain: keep the PE warm with fillers while the final norm
            # chains complete; final biases on the (now idle) scalar queue
            emit_o(*pend.pop(0))
            emit_filler("e0", 2)
            emit_p3(3, 0, 256)
            emit_o(*pend.pop(0))
            emit_filler("e1", 2)
            emit_o(*pend.pop(0))
            emit_filler("d1", 4)
            emit_p3(3, 256, 256, bias_eng="scalar")

    nc.compile()
    return nc


def make_in_maps(x, Wk, Wq, Wv, Wo, bo):
    x = np.asarray(x, np.float32)
    wall = np.concatenate(
        [np.asarray(w, np.float32).T for w in (Wq, Wk, Wv, Wo)], axis=1
    ).astype(NPBF)
    wobc = np.ascontiguousarray(np.asarray(bo, np.float32).reshape(C, 1))
    in_maps = []
    for i in range(NCORES):
        xi = x[BB * i:BB * (i + 1)].reshape(TOK, C)
        in_maps.append({
            "xT": np.ascontiguousarray(xi.T).astype(NPBF),
            "wall": wall, "wobc": wobc,
        })
    return in_maps


_NC_CACHE = None


def kernel(x, Wk, Wq, Wv, Wo, bo):
    global _NC_CACHE
    if _NC_CACHE is None:
        _NC_CACHE = build_module()
    nc = _NC_CACHE
    in_maps = make_in_maps(x, Wk, Wq, Wv, Wo, bo)
    res = run_bass_kernel_spmd(nc, in_maps, core_ids=list(range(NCORES)))
    outs = []
    for i in range(NCORES):
        yt = np.asarray(res.results[i]["yT"]).astype(np.float32)
        outs.append(yt.T.reshape(BB, T, C))
    return np.concatenate(outs, axis=0).astype(np.float32)


# revision 55
# speedup vs baseline: 1.0195x; 1.0195x over previous
"""Multi-head causal self-attention (B=64, T=256, C=384, H=6) on 8 NeuronCores.

Data-parallel over batch: each core processes 8 batches (2048 tokens).
Layouts (all chosen so no device-side transposes are needed):
  - xT, Q.T, K.T feature-major [C, tokens]
  - V token-major [tokens, 128*H]: per head 64 V columns + 64 ones columns,
    so each O.T matmul (stationary [128tok, 128]) produces O.T on PSUM
    partitions 0:64 and the softmax denominator Z replicated on partitions
    64:128 — no separate Z matmuls and no partition-broadcast needed
  - scores computed transposed (S.T[tk, tq]); causal mask via gpsimd
    affine_select on the exp'd probabilities (as in the original module)
  - per pair, one f32 copy evacuates the O PSUM bank to SBUF; 1/Z and the
    normalization then run batched per-batch on vector/gpsimd from SBUF
  - catT merged per 512-token chunk ([128, 1536], feature-chunk-major)
Pipeline: QK-proj(t) / V-proj(t) / scores(u) / O(u-1) interleaved in
emission order (one-stage software pipeline) so the PE stream stays dense;
the out-projection is emitted as soon as its token chunk is normalized.
Matmul operands are bf16; accumulation and softmax denominators fp32;
output is written bf16 (tolerance 2e-2 leaves ample margin) and upcast on
the host.
"""

import sys

import ml_dtypes
import numpy as np

for _p in ("/opt/trn_rl_repo", "/root/.axon_site/_ro/trn_rl_repo"):
    if _p not in sys.path:
        sys.path.insert(0, _p)

import concourse.bass as bass
import concourse.tile as tile
from concourse import bacc, mybir
from concourse.bass_utils import run_bass_kernel_spmd

B, T, C, H, D = 64, 256, 384, 6, 64
NCORES = 8
BB = B // NCORES  # batches per core = 8
TOK = BB * T      # tokens per core = 2048
SCALE = float(C) ** -0.5
F32 = mybir.dt.float32
BF16 = mybir.dt.bfloat16
NPBF = ml_dtypes.bfloat16

NT4 = TOK // 512  # 4 column-chunks of 512 tokens (2 batches each)
NKC = C // 128    # 3 chunks of 128 over feature dim


def build_module():
    nc = bacc.Bacc("TRN2", target_bir_lowering=False, debug=False)

    xT = nc.dram_tensor("xT", [C, TOK], BF16, kind="ExternalInput").ap()
    wall = nc.dram_tensor("wall", [C, 4 * C], BF16, kind="ExternalInput").ap()
    wobc = nc.dram_tensor("wobc", [C, 1], F32, kind="ExternalInput").ap()
    yT = nc.dram_tensor("yT", [C, TOK], BF16, kind="ExternalOutput").ap()

    with tile.TileContext(nc) as tc:
        import contextlib

        ctx = contextlib.ExitStack()
        with ctx:
            consts = ctx.enter_context(tc.tile_pool(name="consts", bufs=1))

            def ptile(name, shape, dt=BF16):
                return consts.tile(shape, dt, tag=name, name=name)

            # ---- persistent SBUF tiles ----
            w_sb = [[ptile(f"w{j}_{k}", [128, C]) for k in range(NKC)]
                    for j in range(4)]  # q, k, v, o
            wq_sb, wk_sb, wv_sb, wo_sb = w_sb
            wob_sb = [ptile(f"wob{k}", [128, 1], F32) for k in range(NKC)]
            xt_sb = [ptile(f"xt{k}", [128, TOK]) for k in range(NKC)]
            qt_sb = [[ptile(f"qt{k}_{t}", [128, 512]) for t in range(NT4)] for k in range(NKC)]
            kt_sb = [[ptile(f"kt{k}_{t}", [128, 512]) for t in range(NT4)] for k in range(NKC)]
            cat_sb = [ptile(f"cat{t}", [128, 3 * 512]) for t in range(NT4)]
            v_sb = [ptile(f"v{t}", [128, 128 * H]) for t in range(2 * BB)]
            warm_sb = ptile("warm", [128, 512])

            # ---- PSUM pools (8 banks: pa 2 + ps 2x2 + po 2) ----
            pa = ctx.enter_context(tc.tile_pool(name="pa", bufs=2, space="PSUM"))
            ps = ctx.enter_context(tc.tile_pool(name="ps", bufs=2, space="PSUM"))
            po = ctx.enter_context(tc.tile_pool(name="po", bufs=2, space="PSUM"))

            # ---- HAM warm-up first (before anything else queues on
            # vector/tensor): dummy matmuls so the PE clock is at 8/8 by the
            # time the first real operands land in SBUF
            nc.gpsimd.memset(warm_sb, 0.0)
            pwarm = pa.tile([128, 512], F32, tag="pa", name="pwarm")
            for i in range(12):
                nc.tensor.matmul(pwarm, warm_sb[:, 0:128], warm_sb,
                                 start=True, stop=True, skip_group_check=True)

            def emit_filler(tag, n=3):
                pf = pa.tile([128, 512], F32, tag="pa", name=f"fill{tag}")
                for i in range(n):
                    nc.tensor.matmul(pf, warm_sb[:, 0:128], warm_sb,
                                     start=True, stop=True,
                                     skip_group_check=True)

            # ---- input DMAs: split fine-grained, ordered so the first QK
            # matmuls can start as early as possible
            dma_list = []
            for j, c0 in ((0, 0), (1, C)):  # wq, wk
                for k in range(NKC):
                    dma_list.append((w_sb[j][k], wall[128 * k:128 * (k + 1), c0:c0 + C]))
            for k in range(NKC):
                dma_list.append((xt_sb[k][:, 0:512], xT[128 * k:128 * (k + 1), 0:512]))
            for k in range(NKC):
                dma_list.append((w_sb[2][k], wall[128 * k:128 * (k + 1), 2 * C:3 * C]))
            for k in range(NKC):
                dma_list.append((xt_sb[k][:, 512:1024], xT[128 * k:128 * (k + 1), 512:1024]))
            for k in range(NKC):
                dma_list.append((w_sb[3][k], wall[128 * k:128 * (k + 1), 3 * C:4 * C]))
            for t in range(2, NT4):
                for k in range(NKC):
                    dma_list.append((
                        xt_sb[k][:, 512 * t:512 * (t + 1)],
                        xT[128 * k:128 * (k + 1), 512 * t:512 * (t + 1)],
                    ))
            for k in range(NKC):
                dma_list.append((wob_sb[k], wobc[128 * k:128 * (k + 1), :]))
            # first-needed 9 transfers strict round-robin (minimize time to
            # first real matmul); the rest weighted sync-heavy
            in_queues = ([nc.sync, nc.gpsimd, nc.scalar] * 3
                         + [nc.sync, nc.gpsimd, nc.sync, nc.scalar,
                            nc.sync, nc.gpsimd, nc.gpsimd, nc.sync] * 3)
            for i, (dst, src) in enumerate(dma_list):
                in_queues[i % len(in_queues)].dma_start(out=dst, in_=src)

            pt_pool = ctx.enter_context(tc.tile_pool(name="ptp", bufs=BB * H // 2))
            ou_pool = ctx.enter_context(tc.tile_pool(name="oup", bufs=3))
            rz_pool = ctx.enter_context(tc.tile_pool(name="rzp", bufs=3))
            y_pool = ctx.enter_context(tc.tile_pool(name="yp", bufs=3))

            # ones columns FIRST in each head's V block: the O matmul then
            # writes Z (replicated) to PSUM partitions 0:64 (base-0, readable
            # by the custom-DVE reciprocal) and O.T to partitions 64:128
            for tb in range(2 * BB):
                vv = v_sb[tb].rearrange("p (h w) -> p h w", w=2 * D)
                nc.gpsimd.memset(vv[:, :, 0:D], 1.0)

            # ---------------- emission helpers ----------------
            def emit_qk(t, co):
                for which, wsb, outsb, eng in (
                    (0, wq_sb, qt_sb, nc.scalar),
                    (1, wk_sb, kt_sb, nc.vector),
                ):
                    pqk = pa.tile([128, 512], F32, tag="pa", name=f"pqk{which}{co}_{t}")
                    for kc in range(NKC):
                        nc.tensor.matmul(
                            pqk,
                            wsb[kc][:, 128 * co:128 * (co + 1)],
                            xt_sb[kc][:, 512 * t:512 * (t + 1)],
                            start=(kc == 0),
                            stop=(kc == NKC - 1),
                        )
                    if eng is nc.scalar:
                        eng.copy(outsb[co][t], pqk)
                    else:
                        eng.tensor_copy(outsb[co][t], pqk)

            def emit_v(tb):
                pv = pa.tile([128, C], F32, tag="pa", name=f"pv{tb}")
                for kc in range(NKC):
                    nc.tensor.matmul(
                        pv,
                        xt_sb[kc][:, 128 * tb:128 * (tb + 1)],
                        wv_sb[kc],
                        start=(kc == 0),
                        stop=(kc == NKC - 1),
                    )
                vv = v_sb[tb].rearrange("p (h w) -> p h w", w=2 * D)
                if tb % 2 == 0:
                    nc.scalar.copy(
                        vv[:, :, D:2 * D], pv.rearrange("p (h d) -> p h d", d=D)
                    )
                else:
                    nc.vector.tensor_copy(
                        vv[:, :, D:2 * D], pv.rearrange("p (h d) -> p h d", d=D)
                    )

            def emit_scores(u):
                b, hp = u
                t, qc = b // 2, (b % 2) * 256
                p_s = ps.tile([128, 1024], F32, tag="ps", name=f"s{b}_{hp}")
                qt, kt = qt_sb[hp][t], kt_sb[hp][t]
                for tkb in range(2):  # interleave heads for row-tile overlap
                    for hh in range(2):
                        r0, s0 = 64 * hh, 512 * hh
                        if tkb == 0:
                            nc.tensor.matmul(
                                p_s[:, s0:s0 + 256],
                                kt[r0:r0 + 64, qc:qc + 128],
                                qt[r0:r0 + 64, qc:qc + 256],
                                start=True, stop=True,
                            )
                        else:
                            nc.tensor.matmul(
                                p_s[:, s0 + 256:s0 + 384],
                                kt[r0:r0 + 64, qc + 128:qc + 256],
                                qt[r0:r0 + 64, qc + 128:qc + 256],
                                start=True, stop=True,
                            )
                # exp (scalar) then causal mask on diagonal blocks (gpsimd)
                pt = pt_pool.tile([128, 768], BF16, tag="pt", name=f"pt{b}_{hp}")
                nc.scalar.activation(
                    pt.rearrange("p (a q) -> p a q", q=384),
                    p_s.rearrange("p (a q) -> p a q", q=512)[:, :, 0:384],
                    mybir.ActivationFunctionType.Exp, scale=SCALE,
                )
                for hh in range(2):
                    sel = pt[:, 384 * hh:384 * (hh + 1)] \
                        .rearrange("p (c i) -> p c i", i=128)[:, 0::2, :]
                    nc.gpsimd.affine_select(
                        out=sel, in_=sel,
                        pattern=[[0, 2], [1, 128]],
                        compare_op=mybir.AluOpType.is_ge,
                        fill=0.0, base=0, channel_multiplier=-1,
                    )
                return pt

            def emit_o(u, pt):
                b, hp = u
                t, qc = b // 2, (b % 2) * 256
                p_o = po.tile([128, 512], F32, tag="po", name=f"po{b}_{hp}")
                for hh in range(2):
                    h = 2 * hp + hh
                    c0 = 256 * hh
                    nc.tensor.matmul(
                        p_o[:, c0:c0 + 256],
                        v_sb[2 * b][:, 128 * h:128 * (h + 1)],
                        pt[:, 384 * hh:384 * hh + 256],
                        start=True, stop=False, skip_group_check=True,
                    )
                    nc.tensor.matmul(
                        p_o[:, c0 + 128:c0 + 256],
                        v_sb[2 * b + 1][:, 128 * h:128 * (h + 1)],
                        pt[:, 384 * hh + 256:384 * hh + 384],
                        start=False, stop=True, skip_group_check=True,
                    )
                # 1/Z straight off PSUM (base-0), O.T rows to SBUF (bf16)
                rz = rz_pool.tile([64, 512], F32, tag="rz", name=f"rz{b}_{hp}")
                nc.vector.reciprocal_approx_fast(rz, p_o[0:64, :])
                ou = ou_pool.tile([64, 512], BF16, tag="ou", name=f"ou{b}_{hp}")
                nc.scalar.copy(ou, p_o[64:128, :])
                # normalize into catT (all-SBUF base-0 inputs; split engines)
                for hh, eng in ((0, nc.gpsimd), (1, nc.vector)):
                    c0 = 256 * hh
                    eng.tensor_mul(
                        cat_sb[t][64 * hh:64 * (hh + 1),
                                  512 * hp + qc:512 * hp + qc + 256],
                        ou[:, c0:c0 + 256],
                        rz[:, c0:c0 + 256],
                    )

            def emit_p3(t, qc0, n, bias_eng="vector"):
                for co in range(NKC):
                    pyk = pa.tile([128, n], F32, tag="pa", name=f"py{co}_{t}_{qc0}")
                    for kc in range(NKC):
                        nc.tensor.matmul(
                            pyk,
                            wo_sb[kc][:, 128 * co:128 * (co + 1)],
                            cat_sb[t][:, 512 * kc + qc0:512 * kc + qc0 + n],
                            start=(kc == 0),
                            stop=(kc == NKC - 1),
                        )
                    yt = y_pool.tile([128, n], BF16, tag="yt", name=f"yt{co}_{t}_{qc0}")
                    if bias_eng == "vector":
                        nc.vector.tensor_scalar_add(yt, pyk, wob_sb[co][:, 0:1])
                    else:
                        nc.scalar.activation(
                            yt, pyk, mybir.ActivationFunctionType.Identity,
                            bias=wob_sb[co][:, 0:1],
                        )
                    q = nc.sync if co < 2 else nc.scalar
                    q.dma_start(
                        out=yT[128 * co:128 * (co + 1),
                               512 * t + qc0:512 * t + qc0 + n],
                        in_=yt,
                    )

            # ---------------- main pipeline ----------------
            units = [(b, hp) for b in range(BB) for hp in range(H // 2)]
            # two-stage software pipeline: O(u) is emitted two units after
            # scores(u), hiding the exp->mask cross-engine latency
            pend = []
            emit_qk(0, 0)
            emit_qk(0, 1)
            for i, u in enumerate(units):
                # scores + exp first (earliest start for the exp->mask chain)
                pt = emit_scores(u)
                # QK chunk for unit i+2 (two units of slack before use) and
                # this unit's share of the V projection
                t, j = i // 6, i % 6
                t2, j2 = (i + 2) // 6, (i + 2) % 6
                if i + 2 < len(units) and j2 < 3:
                    emit_qk(t2, j2)
                if j < 2:
                    emit_v(4 * t + 2 * j)
                    emit_v(4 * t + 2 * j + 1)
                if len(pend) == 3:
                    emit_o(*pend.pop(0))
                # out-proj for chunk t four units after its last norm
                if i >= 11 and (i - 11) % 6 == 0:
                    emit_p3((i - 11) // 6, 0, 512)
                pend.append((u, pt))
            # drain: keep the PE warm with fillers while the final norm
            # chains complete; final biases on the (now idle) scalar queue
            emit_o(*pend.pop(0))
            emit_filler("e0", 2)
            emit_p3(3, 0, 256)
            emit_o(*pend.pop(0))
            emit_filler("e1", 2)
            emit_o(*pend.pop(0))
            emit_filler("d1", 4)
            emit_p3(3, 256, 256, bias_eng="scalar")

    nc.compile()
    return nc


def make_in_maps(x, Wk, Wq, Wv, Wo, bo):
    x = np.asarray(x, np.float32)
    wall = np.concatenate(
        [np.asarray(w, np.float32).T for w in (Wq, Wk, Wv, Wo)], axis=1
    ).astype(NPBF)
    wobc = np.ascontiguousarray(np.asarray(bo, np.float32).reshape(C, 1))
    in_maps = []
    for i in range(NCORES):
        xi = x[BB * i:BB * (i + 1)].reshape(TOK, C)
        in_maps.append({
            "xT": np.ascontiguousarray(xi.T).astype(NPBF),
            "wall": wall, "wobc": wobc,
        })
    return in_maps


_NC_CACHE = None


def kernel(x, Wk, Wq, Wv, Wo, bo):
    global _NC_CACHE
    if _NC_CACHE is None:
        _NC_CACHE = build_module()
    nc = _NC_CACHE
    in_maps = make_in_maps(x, Wk, Wq, Wv, Wo, bo)
    res = run_bass_kernel_spmd(nc, in_maps, core_ids=list(range(NCORES)))
    outs = []
    for i in range(NCORES):
        yt = np.asarray(res.results[i]["yT"]).astype(np.float32)
        outs.append(yt.T.reshape(BB, T, C))
    return np.concatenate(outs, axis=0).astype(np.float32)
